# revision 42
# baseline (speedup 1.0000x reference)
"""Trainium2 Bass kernel for a 16-head causal attention block.

Problem: B=4, S=2048, D_MODEL=2048, N_HEADS=16, D_HEAD=128, fp32 I/O.

Sharding (8 cores): core c handles batch b = c//2 and head-group g = c%2
(8 heads each).  Each core computes its heads' attention and the partial
output projection (sum over its 8 heads) for its batch; the host sums the
two head-group partials per batch and adds the output bias.  No on-device
collectives needed.

Per-core dataflow (fp32 PSUM accumulation everywhere):
  phase 1: v_nat[k, hd] = x @ Vw + vb
    k-block 0 (k<128) in bf16 (those rows feed the short early-q attention
    averages where fp8 noise would not wash out), k-blocks 1..15 via fp8
    DoubleRow (2x PE) with the scale undone in the bias epilogue.
  phase 2 per head h:
    qT/kT = fp8 DoubleRow projections (+bias via DVE / ACT)
    per q-block j (512), k-tile i (128, i <= 4j+3):
      ST[k,q] = kT_tile^T-contract qT block       (PE)
      PT      = exp(ST)  bf16                     (ACT)
      PT     *= causal mask (diagonal tiles)      (DVE, bf16 2x)
      acc    += PT   (bf16 accumulator, 2x DVE)   (DVE)
      attnT  += v_tile^T-contract PT              (PE, PSUM accum)
    denom = partition_all_reduce(acc)             (GPSIMD)
    recip ~ 1/denom                               (DVE)
    attnT_all = attnT * recip  (bf16)             (DVE)
  phase 3: out[p, m] = sum_h attnT_all_h^T-contract Ow_h -> fp32 -> DRAM
    (output DMA issued from the gpsimd SWDGE queue so the SP queue never
    blocks next-rep input prefetches behind phase-3 completion)
"""

import math
import sys

import numpy as np
import ml_dtypes

for _p in ("/opt/trn_rl_repo", "/root/.axon_site/_ro/trn_rl_repo"):
    if _p not in sys.path:
        sys.path.insert(0, _p)

BF16 = ml_dtypes.bfloat16
FP8 = ml_dtypes.float8_e4m3

# fp8(e4m3) DoubleRow scales.  x and the weights are rescaled so fp8 sees
# well-ranged values; the combined scale is undone downstream (softmax exp
# scale for Q/K, bias epilogue for V).
SX = 8.0
SW = 2000.0
SWV = 2000.0

S_FULL = 2048
D_FULL = 2048
NH_LOC = 8  # heads per core
DH = 128
QB = 512  # q block width
N_CORES = 8


def build_program(seq=S_FULL, d_model=D_FULL, n_heads=NH_LOC, loop_n=1):
    import concourse.tile as tile
    from concourse import bacc, bass_isa, mybir

    f32 = mybir.dt.float32
    bf16 = mybir.dt.bfloat16
    fp8 = mybir.dt.float8e4
    AF = mybir.ActivationFunctionType
    DR = mybir.MatmulPerfMode.DoubleRow
    ALU = mybir.AluOpType

    nt = d_model // 128  # contraction (d_model) tiles
    npt = seq // 128  # seq tiles (p / k)
    nqb = seq // QB  # q blocks
    kt_per_qb = QB // 128  # 4
    nhd = n_heads * DH  # concatenated head width
    nblk = nhd // 512  # 512-wide chunks of (h, d)

    nc = bacc.Bacc(
        "TRN2", target_bir_lowering=False, debug=False, enable_asserts=False
    )

    exp_scale = 1.0 / ((SX * SW) ** 2 * math.sqrt(DH))
    v_descale = 1.0 / (SX * SWV)

    xt8_d = nc.dram_tensor("xt8", [128, nt, seq], fp8, kind="ExternalInput").ap()
    # bf16 V weights with xT's first 128 columns appended per m-tile: one
    # tensor/DMA so the xT slice shares vwb's early WAR gate instead of
    # sitting at the top of the SBUF stack where it collides with the
    # previous rep's phase-3 store buffers
    vwk_d = nc.dram_tensor("vwk", [128, nt, nhd + 128], bf16, kind="ExternalInput").ap()
    vw8_d = nc.dram_tensor("vw8", [128, nt, nhd], fp8, kind="ExternalInput").ap()
    qw_d = nc.dram_tensor("qw", [128, n_heads, nt, 128], fp8, kind="ExternalInput").ap()
    kw_d = nc.dram_tensor("kw", [128, n_heads, nt, 128], fp8, kind="ExternalInput").ap()
    ow_d = nc.dram_tensor("ow", [128, n_heads, d_model], bf16, kind="ExternalInput").ap()
    qb_d = nc.dram_tensor("qb", [128, n_heads], f32, kind="ExternalInput").ap()
    kb_d = nc.dram_tensor("kb", [128, n_heads], f32, kind="ExternalInput").ap()
    vb_d = nc.dram_tensor("vb", [128, nhd], bf16, kind="ExternalInput").ap()
    mask_d = nc.dram_tensor("mask", [128, 512], bf16, kind="ExternalInput").ap()
    out_d = nc.dram_tensor("out", [seq, d_model], f32, kind="ExternalOutput").ap()

    from concourse import library_config

    with tile.TileContext(nc) as tc:
        nc.gpsimd.load_library(library_config.attn)
        for _rep in range(loop_n):
            # PSUM pools: 2 + 2 + 4 = 8 banks.  mm last so phase 3 can pop it
            # and re-use its banks for 2-bank-wide output tiles whose WAR
            # resolves at the h7 QK copies, not the h7 exp tail
            st_ps = tc.alloc_tile_pool(name="stps", bufs=2, space="PSUM")
            pv_ps = tc.alloc_tile_pool(name="pvps", bufs=2, space="PSUM")
            mm_ps = tc.alloc_tile_pool(name="mmps", bufs=4, space="PSUM")

            consts = tc.alloc_tile_pool(name="consts", bufs=1)
            xt8_pool = tc.alloc_tile_pool(name="xt8p", bufs=1)
            vall_pool = tc.alloc_tile_pool(name="vallp", bufs=1)
            att_pool = tc.alloc_tile_pool(
                name="attp", bufs=n_heads * (seq // QB), side="right"
            )

            mask_sb = consts.tile([128, 512], bf16)
            qb_sb = consts.tile([128, n_heads], f32)
            kb_sb = consts.tile([128, n_heads], f32)
            vb_sb = consts.tile([128, nhd], bf16)

            xt8 = xt8_pool.tile([128, nt, seq], fp8)
            vall = vall_pool.tile([128, npt, nhd], bf16)
            # per-(head, q-block) attn tiles keep phase-3 deps fine-grained:
            # O-proj on a p-block waits only on that q-block's normalize, not
            # on the last head's last softmax chain
            atts = [
                [
                    att_pool.tile([128, QB], bf16, tag="att", name=f"att_{h}_{j}")
                    for j in range(nqb)
                ]
                for h in range(n_heads)
            ]

            # ---------------- phase 1a: bf16 v for k-block 0 ----------------
            # vw8 gets its own stack slot BELOW vwk: if it reused vwk's space,
            # its load would WAR-wait on phase-1a's reads of vwk, which chain
            # behind the previous rep's phase 3 on the in-order PE
            vw8_pool = tc.alloc_tile_pool(name="vw8p", bufs=1)
            vwk_pool = tc.alloc_tile_pool(name="vwkp", bufs=1)
            vw8 = vw8_pool.tile([128, nt, nhd], fp8)
            vwk = vwk_pool.tile([128, nt, nhd + 128], bf16)
            nc.sync.dma_start(vwk, vwk_d)
            nc.sync.dma_start(xt8, xt8_d)
            nc.sync.dma_start(vw8, vw8_d)
            # consts via the ACT HWDGE queue: their buffers recycle at rep end,
            # so on the SP queue they would head-of-line-block the next rep's
            # big input streams
            nc.scalar.dma_start(vb_sb, vb_d)
            nc.scalar.dma_start(mask_sb, mask_d)
            nc.scalar.dma_start(qb_sb, qb_d)
            nc.scalar.dma_start(kb_sb, kb_d)

            p1_pools = [mm_ps, st_ps, pv_ps]
            p1_tags = ["mm", "st", "pv"]
            for blk in range(nblk):
                vps = p1_pools[blk].tile(
                    [128, 512], f32, tag=p1_tags[blk], name=f"vps0_{blk}"
                )
                for m in range(nt):
                    nc.tensor.matmul(
                        vps,
                        vwk[:, m, nhd : nhd + 128],
                        vwk[:, m, blk * 512 : (blk + 1) * 512],
                        start=(m == 0),
                        stop=(m == nt - 1),
                    )
                nc.vector.tensor_add(
                    vall[:, 0, blk * 512 : (blk + 1) * 512],
                    vps,
                    vb_sb[:, blk * 512 : (blk + 1) * 512],
                )
            vwk_pool.release()

            # -------------- phase 1b: fp8 DoubleRow v for k >= 128 ----------
            for p_i in range(1, npt):
                for blk in range(nblk):
                    pidx = (p_i * nblk + blk) % 3
                    vps = p1_pools[pidx].tile(
                        [128, 512], f32, tag=p1_tags[pidx], name=f"vps_{p_i}_{blk}"
                    )
                    for m2 in range(nt // 2):
                        nc.tensor.matmul(
                            vps,
                            xt8[:, 2 * m2 : 2 * m2 + 2, p_i * 128 : (p_i + 1) * 128],
                            vw8[:, 2 * m2 : 2 * m2 + 2, blk * 512 : (blk + 1) * 512],
                            start=(m2 == 0),
                            stop=(m2 == nt // 2 - 1),
                            perf_mode=DR,
                        )
                    nc.vector.scalar_tensor_tensor(
                        vall[:, p_i, blk * 512 : (blk + 1) * 512],
                        vps,
                        v_descale,
                        vb_sb[:, blk * 512 : (blk + 1) * 512],
                        op0=ALU.mult,
                        op1=ALU.add,
                    )
            vw8_pool.release()

            # ---------------- phase 2: per-head attention ----------------
            # ow prefetch into the space vwb/vw8 vacated; ACT's HWDGE queue
            # so the SP queue stays clear for next-rep input streams
            ow_pool = tc.alloc_tile_pool(name="owp", bufs=1, side="right")
            ow_all = ow_pool.tile([128, n_heads, d_model], bf16)
            nc.scalar.dma_start(ow_all, ow_d)

            w_pool = tc.alloc_tile_pool(name="wp", bufs=4)
            qk_pool = tc.alloc_tile_pool(name="qkp", bufs=2)
            pt_pool = tc.alloc_tile_pool(name="ptp", bufs=8)
            acc_pool = tc.alloc_tile_pool(name="accp", bufs=3)
            rc_pool = tc.alloc_tile_pool(name="rcp", bufs=2)
            rb_pool = tc.alloc_tile_pool(name="rbp", bufs=2)

            for h in range(n_heads):
                # per-head weight loads; the SP queue runs a head ahead of
                # the PE (bufs=4 double-buffers wq/wk), prefetching naturally
                wq = w_pool.tile([128, nt, 128], fp8, tag="wq", name=f"wq_{h}")
                nc.sync.dma_start(wq, qw_d[:, h])
                wk = w_pool.tile([128, nt, 128], fp8, tag="wk", name=f"wk_{h}")
                nc.sync.dma_start(wk, kw_d[:, h])
                qT = qk_pool.tile([128, seq], bf16, tag="qT", name=f"qT_{h}")
                kT = qk_pool.tile([128, seq], bf16, tag="kT", name=f"kT_{h}")
                for pb in range(seq // 512):
                    qps = mm_ps.tile([128, 512], f32, tag="mm", name=f"qps_{h}_{pb}")
                    for m2 in range(nt // 2):
                        nc.tensor.matmul(
                            qps,
                            wq[:, 2 * m2 : 2 * m2 + 2, :],
                            xt8[:, 2 * m2 : 2 * m2 + 2, pb * 512 : (pb + 1) * 512],
                            start=(m2 == 0),
                            stop=(m2 == nt // 2 - 1),
                            perf_mode=DR,
                        )
                    # Q bias epilogue on DVE (ACT is the phase-2 bottleneck)
                    nc.vector.tensor_scalar_add(
                        qT[:, pb * 512 : (pb + 1) * 512], qps, qb_sb[:, h : h + 1]
                    )
                    kps = mm_ps.tile([128, 512], f32, tag="mm", name=f"kps_{h}_{pb}")
                    for m2 in range(nt // 2):
                        nc.tensor.matmul(
                            kps,
                            wk[:, 2 * m2 : 2 * m2 + 2, :],
                            xt8[:, 2 * m2 : 2 * m2 + 2, pb * 512 : (pb + 1) * 512],
                            start=(m2 == 0),
                            stop=(m2 == nt // 2 - 1),
                            perf_mode=DR,
                        )
                    nc.scalar.activation(
                        kT[:, pb * 512 : (pb + 1) * 512],
                        kps,
                        AF.Identity,
                        bias=kb_sb[:, h : h + 1],
                    )
                for j in range(nqb):
                    nk = (j + 1) * kt_per_qb
                    aps = pv_ps.tile([128, 512], f32, tag="pv", name=f"aps_{h}_{j}")
                    acc = acc_pool.tile([128, 512], bf16, tag="acc", name=f"acc_{h}_{j}")
                    for i in range(nk):
                        # diagonal-crossing tiles: only columns c >= s are
                        # causally valid; skip the dead prefix entirely
                        s = 128 * (i - kt_per_qb * j) if i >= kt_per_qb * j else 0
                        w = 512 - s
                        stp = st_ps.tile([128, 512], f32, tag="st", name=f"stp_{h}_{j}_{i}")
                        nc.tensor.matmul(
                            stp[:, s:512],
                            kT[:, i * 128 : (i + 1) * 128],
                            qT[:, j * 512 + s : (j + 1) * 512],
                            start=True,
                            stop=True,
                        )
                        ptile = pt_pool.tile(
                            [128, 512], bf16, tag="pt", name=f"pt_{h}_{j}_{i}"
                        )
                        nc.scalar.activation(
                            ptile[:, 0:w], stp[:, s:512], AF.Exp, scale=exp_scale
                        )
                        if i >= kt_per_qb * j:
                            nc.vector.tensor_mul(
                                ptile[:, 0:w], ptile[:, 0:w], mask_sb[:, 0 : 512 - s]
                            )
                        # for j>=1 tiles i=0,1 are both full width: fuse the
                        # copy+add seed into one DVE add of the two PT tiles
                        if i == 0:
                            pt_first = ptile
                        elif i == 1:
                            if s > 0:
                                nc.vector.tensor_copy(
                                    acc[:, 0:s], pt_first[:, 0:s]
                                )
                                nc.vector.tensor_add(
                                    acc[:, s:512], pt_first[:, s:512], ptile[:, 0:w]
                                )
                            else:
                                nc.vector.tensor_add(acc, pt_first, ptile)
                        else:
                            nc.vector.tensor_add(
                                acc[:, s:512], acc[:, s:512], ptile[:, 0:w]
                            )
                        nc.tensor.matmul(
                            aps[:, s:512],
                            vall[:, i, h * 128 : (h + 1) * 128],
                            ptile[:, 0:w],
                            start=(i == 0),
                            stop=(i == nk - 1),
                        )
                    dnr = rb_pool.tile([128, 512], f32, tag="rb", name=f"dnr_{h}_{j}")
                    nc.gpsimd.partition_all_reduce(
                        dnr, acc, 128, bass_isa.ReduceOp.add
                    )
                    rc = rc_pool.tile([128, 512], f32, tag="rc", name=f"rc_{h}_{j}")
                    nc.vector.reciprocal_approx_fast(rc, dnr)
                    nc.vector.tensor_mul(atts[h][j], aps, rc)
            rb_pool.release()
            rc_pool.release()
            acc_pool.release()
            pt_pool.release()
            qk_pool.release()
            w_pool.release()
            vall_pool.release()
            xt8_pool.release()
            # consts sits at the bottom of the left stack: releasing it here
            # (its last reader is phase 2) lets the next rep's entire left
            # stack allocate — and its input DMAs start — during phase 3
            consts.release()

            # ---------------- phase 3: output projection ----------------
            osb_pool = tc.alloc_tile_pool(name="osbp", bufs=3, side="right")
            for p_i in range(npt):
                osb = osb_pool.tile([128, d_model], f32, tag="osb", name=f"osb_{p_i}")
                for mb in range(d_model // 512):
                    ops = mm_ps.tile([128, 512], f32, tag="mm", name=f"ops_{p_i}_{mb}")
                    for h in range(n_heads):
                        nc.tensor.matmul(
                            ops,
                            atts[h][p_i // kt_per_qb][
                                :, (p_i % kt_per_qb) * 128 : (p_i % kt_per_qb + 1) * 128
                            ],
                            ow_all[:, h, mb * 512 : (mb + 1) * 512],
                            start=(h == 0),
                            stop=(h == n_heads - 1),
                        )
                    nc.scalar.copy(osb[:, mb * 512 : (mb + 1) * 512], ops)
                # one batched row-block store via the gpsimd SWDGE queue
                nc.gpsimd.dma_start(
                    out_d[p_i * 128 : (p_i + 1) * 128, :], osb
                )
            osb_pool.release()
            ow_pool.release()
            att_pool.release()
            mm_ps.release()
            pv_ps.release()
            st_ps.release()
    nc.finalize()
    return nc


def make_core_inputs(x_b, Qw, Qb, Kw, Kb, Vw, Vb, Ow, seq, d_model, n_heads):
    """Host-side prep of one core's input map.

    x_b: [seq, d_model] fp32.  Qw/Kw/Vw: [n_heads, d_model, 128].
    Qb/Kb/Vb: [n_heads, 128].  Ow: [n_heads, 128, d_model].
    """
    nt = d_model // 128
    nhd = n_heads * DH

    # xT as [128(m_in), nt, seq]
    xTr = x_b.T.reshape(nt, 128, seq).transpose(1, 0, 2)
    xt8 = np.ascontiguousarray((xTr * SX).astype(FP8))
    qw = np.ascontiguousarray(
        (Qw * SW).reshape(n_heads, nt, 128, 128).transpose(2, 0, 1, 3).astype(FP8)
    )
    kw = np.ascontiguousarray(
        (Kw * SW).reshape(n_heads, nt, 128, 128).transpose(2, 0, 1, 3).astype(FP8)
    )
    # v weights as [128(m_in), nt, (h d)]
    vw_nat = (
        Vw.transpose(1, 0, 2)
        .reshape(d_model, nhd)
        .reshape(nt, 128, nhd)
        .transpose(1, 0, 2)
    )
    vwk = np.ascontiguousarray(
        np.concatenate([vw_nat, xTr[:, :, 0:128]], axis=2).astype(BF16)
    )
    vw8 = np.ascontiguousarray((vw_nat * SWV).astype(FP8))
    ow = np.ascontiguousarray(Ow.transpose(1, 0, 2).astype(BF16))
    qb = np.ascontiguousarray((Qb * SX * SW).T.astype(np.float32))
    kb = np.ascontiguousarray((Kb * SX * SW).T.astype(np.float32))
    vb = np.ascontiguousarray(
        np.broadcast_to(Vb.reshape(1, nhd), (128, nhd)).astype(BF16)
    )
    r = np.arange(128, dtype=np.int64)[:, None]
    u = np.arange(512, dtype=np.int64)[None, :]
    mask = (r <= u).astype(BF16)
    return {
        "xt8": xt8,
        "vwk": vwk,
        "vw8": vw8,
        "qw": qw,
        "kw": kw,
        "ow": ow,
        "qb": qb,
        "kb": kb,
        "vb": vb,
        "mask": mask,
    }


_NC_CACHE = None


def kernel(**inputs):
    global _NC_CACHE
    from concourse.bass_utils import run_bass_kernel_spmd

    x = np.asarray(inputs["x"], np.float32)
    Q_w = np.asarray(inputs["Q_w"], np.float32)
    Q_b = np.asarray(inputs["Q_b"], np.float32)
    K_w = np.asarray(inputs["K_w"], np.float32)
    K_b = np.asarray(inputs["K_b"], np.float32)
    V_w = np.asarray(inputs["V_w"], np.float32)
    V_b = np.asarray(inputs["V_b"], np.float32)
    O_w = np.asarray(inputs["O_w"], np.float32)
    O_b = np.asarray(inputs["O_b"], np.float32)

    B, seq, d_model = x.shape

    if _NC_CACHE is None:
        _NC_CACHE = build_program(seq=seq, d_model=d_model, n_heads=NH_LOC)
    nc = _NC_CACHE

    in_maps = []
    for c in range(N_CORES):
        b = c // 2
        g = c % 2
        hs = slice(g * NH_LOC, (g + 1) * NH_LOC)
        in_maps.append(
            make_core_inputs(
                x[b], Q_w[hs], Q_b[hs], K_w[hs], K_b[hs], V_w[hs], V_b[hs],
                O_w[hs], seq, d_model, NH_LOC,
            )
        )

    res = run_bass_kernel_spmd(nc, in_maps, core_ids=list(range(N_CORES)))
    out = np.empty((B, seq, d_model), np.float32)
    for b in range(B):
        out[b] = res.results[2 * b]["out"] + res.results[2 * b + 1]["out"] + O_b[None, :]
    return out


# revision 46
# speedup vs baseline: 1.0086x; 1.0086x over previous
"""Trainium2 Bass kernel for a 16-head causal attention block.

Problem: B=4, S=2048, D_MODEL=2048, N_HEADS=16, D_HEAD=128, fp32 I/O.

Sharding (8 cores): core c handles batch b = c//2 and head-group g = c%2
(8 heads each).  Each core computes its heads' attention and the partial
output projection (sum over its 8 heads) for its batch; the host sums the
two head-group partials per batch and adds the output bias.  No on-device
collectives needed.

Per-core dataflow (fp32 PSUM accumulation everywhere):
  phase 1: v_nat[k, hd] = x @ Vw + vb
    k-block 0 (k<128) in bf16 (those rows feed the short early-q attention
    averages where fp8 noise would not wash out), k-blocks 1..15 via fp8
    DoubleRow (2x PE) with the scale undone in the bias epilogue.
  phase 2 per head h:
    qT/kT = fp8 DoubleRow projections (+bias via DVE / ACT)
    per q-block j (512), k-tile i (128, i <= 4j+3):
      ST[k,q] = kT_tile^T-contract qT block       (PE)
      PT      = exp(ST)  bf16                     (ACT)
      PT     *= causal mask (diagonal tiles)      (DVE, bf16 2x)
      acc    += PT   (bf16 accumulator, 2x DVE)   (DVE)
      attnT  += v_tile^T-contract PT              (PE, PSUM accum)
    denom = partition_all_reduce(acc)             (GPSIMD)
    recip ~ 1/denom                               (DVE)
    attnT_all = attnT * recip  (bf16)             (DVE)
  phase 3: out[p, m] = sum_h attnT_all_h^T-contract Ow_h -> fp32 -> DRAM
    (output DMA issued from the gpsimd SWDGE queue so the SP queue never
    blocks next-rep input prefetches behind phase-3 completion)
"""

import math
import sys

import numpy as np
import ml_dtypes

for _p in ("/opt/trn_rl_repo", "/root/.axon_site/_ro/trn_rl_repo"):
    if _p not in sys.path:
        sys.path.insert(0, _p)

BF16 = ml_dtypes.bfloat16
FP8 = ml_dtypes.float8_e4m3

# fp8(e4m3) DoubleRow scales.  x and the weights are rescaled so fp8 sees
# well-ranged values; the combined scale is undone downstream (softmax exp
# scale for Q/K, bias epilogue for V).
SX = 8.0
SW = 2000.0
SWV = 2000.0

S_FULL = 2048
D_FULL = 2048
NH_LOC = 8  # heads per core
DH = 128
QB = 512  # q block width
N_CORES = 8


def build_program(seq=S_FULL, d_model=D_FULL, n_heads=NH_LOC, loop_n=1):
    import concourse.tile as tile
    from concourse import bacc, bass_isa, mybir

    f32 = mybir.dt.float32
    bf16 = mybir.dt.bfloat16
    fp8 = mybir.dt.float8e4
    AF = mybir.ActivationFunctionType
    DR = mybir.MatmulPerfMode.DoubleRow
    ALU = mybir.AluOpType

    nt = d_model // 128  # contraction (d_model) tiles
    npt = seq // 128  # seq tiles (p / k)
    nqb = seq // QB  # q blocks
    kt_per_qb = QB // 128  # 4
    nhd = n_heads * DH  # concatenated head width
    nblk = nhd // 512  # 512-wide chunks of (h, d)

    nc = bacc.Bacc(
        "TRN2", target_bir_lowering=False, debug=False, enable_asserts=False
    )

    exp_scale = 1.0 / ((SX * SW) ** 2 * math.sqrt(DH))
    v_descale = 1.0 / (SX * SWV)

    xt8_d = nc.dram_tensor("xt8", [128, nt, seq], fp8, kind="ExternalInput").ap()
    # bf16 V weights with xT's first 128 columns appended per m-tile: one
    # tensor/DMA so the xT slice shares vwb's early WAR gate instead of
    # sitting at the top of the SBUF stack where it collides with the
    # previous rep's phase-3 store buffers
    vwk_d = nc.dram_tensor("vwk", [128, nt, nhd + 128], bf16, kind="ExternalInput").ap()
    vw8_d = nc.dram_tensor("vw8", [128, nt, nhd], fp8, kind="ExternalInput").ap()
    qw_d = nc.dram_tensor("qw", [128, n_heads, nt, 128], fp8, kind="ExternalInput").ap()
    kw_d = nc.dram_tensor("kw", [128, n_heads, nt, 128], fp8, kind="ExternalInput").ap()
    ow_d = nc.dram_tensor("ow", [128, n_heads, d_model], bf16, kind="ExternalInput").ap()
    qb_d = nc.dram_tensor("qb", [128, n_heads], f32, kind="ExternalInput").ap()
    kb_d = nc.dram_tensor("kb", [128, n_heads], f32, kind="ExternalInput").ap()
    vb_d = nc.dram_tensor("vb", [128, nhd], bf16, kind="ExternalInput").ap()
    mask_d = nc.dram_tensor("mask", [128, 512], bf16, kind="ExternalInput").ap()
    out_d = nc.dram_tensor("out", [seq, d_model], f32, kind="ExternalOutput").ap()

    from concourse import library_config

    with tile.TileContext(nc) as tc:
        nc.gpsimd.load_library(library_config.attn)
        for _rep in range(loop_n):
            # PSUM pools: 4 + 2 + 2 = 8 banks.  st holds 2-bank score-tile
            # PAIRS: one exp covers two k-tiles, halving ACT instruction
            # count on the full tiles and doubling score lookahead
            st_ps = tc.alloc_tile_pool(name="stps", bufs=2, space="PSUM")
            pv_ps = tc.alloc_tile_pool(name="pvps", bufs=2, space="PSUM")
            mm_ps = tc.alloc_tile_pool(name="mmps", bufs=2, space="PSUM")

            consts = tc.alloc_tile_pool(name="consts", bufs=1)
            xt8_pool = tc.alloc_tile_pool(name="xt8p", bufs=1)
            vall_pool = tc.alloc_tile_pool(name="vallp", bufs=1)
            att_pool = tc.alloc_tile_pool(
                name="attp", bufs=n_heads * (seq // QB), side="right"
            )

            mask_sb = consts.tile([128, 512], bf16)
            qb_sb = consts.tile([128, n_heads], f32)
            kb_sb = consts.tile([128, n_heads], f32)
            vb_sb = consts.tile([128, nhd], bf16)

            xt8 = xt8_pool.tile([128, nt, seq], fp8)
            vall = vall_pool.tile([128, npt, nhd], bf16)
            # per-(head, q-block) attn tiles keep phase-3 deps fine-grained:
            # O-proj on a p-block waits only on that q-block's normalize, not
            # on the last head's last softmax chain
            atts = [
                [
                    att_pool.tile([128, QB], bf16, tag="att", name=f"att_{h}_{j}")
                    for j in range(nqb)
                ]
                for h in range(n_heads)
            ]

            # ---------------- phase 1a: bf16 v for k-block 0 ----------------
            # vw8 gets its own stack slot BELOW vwk: if it reused vwk's space,
            # its load would WAR-wait on phase-1a's reads of vwk, which chain
            # behind the previous rep's phase 3 on the in-order PE
            vw8_pool = tc.alloc_tile_pool(name="vw8p", bufs=1)
            vwk_pool = tc.alloc_tile_pool(name="vwkp", bufs=1)
            vw8 = vw8_pool.tile([128, nt, nhd], fp8)
            vwk = vwk_pool.tile([128, nt, nhd + 128], bf16)
            nc.sync.dma_start(vwk, vwk_d)
            nc.sync.dma_start(xt8, xt8_d)
            nc.sync.dma_start(vw8, vw8_d)
            # consts via the ACT HWDGE queue: their buffers recycle at rep end,
            # so on the SP queue they would head-of-line-block the next rep's
            # big input streams
            nc.scalar.dma_start(vb_sb, vb_d)
            nc.scalar.dma_start(mask_sb, mask_d)
            nc.scalar.dma_start(qb_sb, qb_d)
            nc.scalar.dma_start(kb_sb, kb_d)

            p1_pools = [mm_ps, pv_ps]
            p1_tags = ["mm", "pv"]
            for blk in range(nblk):
                vps = p1_pools[blk].tile(
                    [128, 512], f32, tag=p1_tags[blk], name=f"vps0_{blk}"
                )
                for m in range(nt):
                    nc.tensor.matmul(
                        vps,
                        vwk[:, m, nhd : nhd + 128],
                        vwk[:, m, blk * 512 : (blk + 1) * 512],
                        start=(m == 0),
                        stop=(m == nt - 1),
                    )
                nc.vector.tensor_add(
                    vall[:, 0, blk * 512 : (blk + 1) * 512],
                    vps,
                    vb_sb[:, blk * 512 : (blk + 1) * 512],
                )
            vwk_pool.release()

            # -------------- phase 1b: fp8 DoubleRow v for k >= 128 ----------
            for p_i in range(1, npt):
                for blk in range(nblk):
                    pidx = (p_i * nblk + blk) % 2
                    vps = p1_pools[pidx].tile(
                        [128, 512], f32, tag=p1_tags[pidx], name=f"vps_{p_i}_{blk}"
                    )
                    for m2 in range(nt // 2):
                        nc.tensor.matmul(
                            vps,
                            xt8[:, 2 * m2 : 2 * m2 + 2, p_i * 128 : (p_i + 1) * 128],
                            vw8[:, 2 * m2 : 2 * m2 + 2, blk * 512 : (blk + 1) * 512],
                            start=(m2 == 0),
                            stop=(m2 == nt // 2 - 1),
                            perf_mode=DR,
                        )
                    nc.vector.scalar_tensor_tensor(
                        vall[:, p_i, blk * 512 : (blk + 1) * 512],
                        vps,
                        v_descale,
                        vb_sb[:, blk * 512 : (blk + 1) * 512],
                        op0=ALU.mult,
                        op1=ALU.add,
                    )
            vw8_pool.release()

            # ---------------- phase 2: per-head attention ----------------
            # ow prefetch into the space vwb/vw8 vacated; ACT's HWDGE queue
            # so the SP queue stays clear for next-rep input streams
            ow_pool = tc.alloc_tile_pool(name="owp", bufs=1, side="right")
            ow_all = ow_pool.tile([128, n_heads, d_model], bf16)
            nc.scalar.dma_start(ow_all, ow_d)

            w_pool = tc.alloc_tile_pool(name="wp", bufs=4)
            qk_pool = tc.alloc_tile_pool(name="qkp", bufs=2)
            pt_pool = tc.alloc_tile_pool(name="ptp", bufs=8)
            acc_pool = tc.alloc_tile_pool(name="accp", bufs=3)
            rc_pool = tc.alloc_tile_pool(name="rcp", bufs=2)
            rb_pool = tc.alloc_tile_pool(name="rbp", bufs=2)

            for h in range(n_heads):
                # per-head weight loads; the SP queue runs a head ahead of
                # the PE (bufs=4 double-buffers wq/wk), prefetching naturally
                wq = w_pool.tile([128, nt, 128], fp8, tag="wq", name=f"wq_{h}")
                nc.sync.dma_start(wq, qw_d[:, h])
                wk = w_pool.tile([128, nt, 128], fp8, tag="wk", name=f"wk_{h}")
                nc.sync.dma_start(wk, kw_d[:, h])
                qT = qk_pool.tile([128, seq], bf16, tag="qT", name=f"qT_{h}")
                kT = qk_pool.tile([128, seq], bf16, tag="kT", name=f"kT_{h}")
                for pb in range(seq // 512):
                    qps = mm_ps.tile([128, 512], f32, tag="mm", name=f"qps_{h}_{pb}")
                    for m2 in range(nt // 2):
                        nc.tensor.matmul(
                            qps,
                            wq[:, 2 * m2 : 2 * m2 + 2, :],
                            xt8[:, 2 * m2 : 2 * m2 + 2, pb * 512 : (pb + 1) * 512],
                            start=(m2 == 0),
                            stop=(m2 == nt // 2 - 1),
                            perf_mode=DR,
                        )
                    # Q bias epilogue on DVE (ACT is the phase-2 bottleneck)
                    nc.vector.tensor_scalar_add(
                        qT[:, pb * 512 : (pb + 1) * 512], qps, qb_sb[:, h : h + 1]
                    )
                    kps = mm_ps.tile([128, 512], f32, tag="mm", name=f"kps_{h}_{pb}")
                    for m2 in range(nt // 2):
                        nc.tensor.matmul(
                            kps,
                            wk[:, 2 * m2 : 2 * m2 + 2, :],
                            xt8[:, 2 * m2 : 2 * m2 + 2, pb * 512 : (pb + 1) * 512],
                            start=(m2 == 0),
                            stop=(m2 == nt // 2 - 1),
                            perf_mode=DR,
                        )
                    nc.scalar.activation(
                        kT[:, pb * 512 : (pb + 1) * 512],
                        kps,
                        AF.Identity,
                        bias=kb_sb[:, h : h + 1],
                    )
                for j in range(nqb):
                    nk = (j + 1) * kt_per_qb
                    nfull = kt_per_qb * j
                    aps = pv_ps.tile([128, 512], f32, tag="pv", name=f"aps_{h}_{j}")
                    acc = acc_pool.tile([128, 512], bf16, tag="acc", name=f"acc_{h}_{j}")
                    # full k-tiles in pairs sharing a 2-bank score tile: two
                    # matmuls, ONE exp over [128, 1024]
                    for p in range(nfull // 2):
                        i0 = 2 * p
                        stp = st_ps.tile(
                            [128, 2, 512], f32, tag="st", name=f"stp_{h}_{j}_f{p}"
                        )
                        for half in range(2):
                            nc.tensor.matmul(
                                stp[:, half, :],
                                kT[:, (i0 + half) * 128 : (i0 + half + 1) * 128],
                                qT[:, j * 512 : (j + 1) * 512],
                                start=True,
                                stop=True,
                            )
                        pt2 = pt_pool.tile(
                            [128, 2, 512], bf16, tag="pt", name=f"pt_{h}_{j}_f{p}"
                        )
                        nc.scalar.activation(pt2, stp, AF.Exp, scale=exp_scale)
                        if p == 0:
                            nc.vector.tensor_add(acc, pt2[:, 0, :], pt2[:, 1, :])
                        else:
                            nc.vector.tensor_add(acc, acc, pt2[:, 0, :])
                            nc.vector.tensor_add(acc, acc, pt2[:, 1, :])
                        for half in range(2):
                            nc.tensor.matmul(
                                aps,
                                vall[:, i0 + half, h * 128 : (h + 1) * 128],
                                pt2[:, half, :],
                                start=(i0 + half == 0),
                                stop=False,
                            )
                    # diagonal staircase tiles: only columns c >= s are
                    # causally valid; skip the dead prefix entirely
                    for t in range(kt_per_qb):
                        i = nfull + t
                        s = 128 * t
                        w = 512 - s
                        stp = st_ps.tile(
                            [128, 2, 512], f32, tag="st", name=f"stp_{h}_{j}_s{t}"
                        )
                        nc.tensor.matmul(
                            stp[:, 0, 0:w],
                            kT[:, i * 128 : (i + 1) * 128],
                            qT[:, j * 512 + s : (j + 1) * 512],
                            start=True,
                            stop=True,
                        )
                        ptile = pt_pool.tile(
                            [128, 2, 512], bf16, tag="pt", name=f"pt_{h}_{j}_s{t}"
                        )
                        nc.scalar.activation(
                            ptile[:, 0, 0:w], stp[:, 0, 0:w], AF.Exp, scale=exp_scale
                        )
                        nc.vector.tensor_mul(
                            ptile[:, 0, 0:w], ptile[:, 0, 0:w], mask_sb[:, 0:w]
                        )
                        if j == 0:
                            # no full tiles: seed acc from the staircase
                            if t == 0:
                                pt_first = ptile
                            elif t == 1:
                                nc.vector.tensor_copy(
                                    acc[:, 0:s], pt_first[:, 0, 0:s]
                                )
                                nc.vector.tensor_add(
                                    acc[:, s:512], pt_first[:, 0, s:512],
                                    ptile[:, 0, 0:w],
                                )
                            else:
                                nc.vector.tensor_add(
                                    acc[:, s:512], acc[:, s:512], ptile[:, 0, 0:w]
                                )
                        else:
                            nc.vector.tensor_add(
                                acc[:, s:512], acc[:, s:512], ptile[:, 0, 0:w]
                            )
                        nc.tensor.matmul(
                            aps[:, s:512],
                            vall[:, i, h * 128 : (h + 1) * 128],
                            ptile[:, 0, 0:w],
                            start=(i == 0),
                            stop=(i == nk - 1),
                        )
                    dnr = rb_pool.tile([128, 512], f32, tag="rb", name=f"dnr_{h}_{j}")
                    nc.gpsimd.partition_all_reduce(
                        dnr, acc, 128, bass_isa.ReduceOp.add
                    )
                    rc = rc_pool.tile([128, 512], f32, tag="rc", name=f"rc_{h}_{j}")
                    nc.vector.reciprocal_approx_fast(rc, dnr)
                    nc.vector.tensor_mul(atts[h][j], aps, rc)
            rb_pool.release()
            rc_pool.release()
            acc_pool.release()
            pt_pool.release()
            qk_pool.release()
            w_pool.release()
            vall_pool.release()
            xt8_pool.release()
            # consts sits at the bottom of the left stack: releasing it here
            # (its last reader is phase 2) lets the next rep's entire left
            # stack allocate — and its input DMAs start — during phase 3
            consts.release()

            # ---------------- phase 3: output projection ----------------
            osb_pool = tc.alloc_tile_pool(name="osbp", bufs=3, side="right")
            for p_i in range(npt):
                osb = osb_pool.tile([128, d_model], f32, tag="osb", name=f"osb_{p_i}")
                for mb in range(d_model // 512):
                    ops = mm_ps.tile([128, 512], f32, tag="mm", name=f"ops_{p_i}_{mb}")
                    for h in range(n_heads):
                        nc.tensor.matmul(
                            ops,
                            atts[h][p_i // kt_per_qb][
                                :, (p_i % kt_per_qb) * 128 : (p_i % kt_per_qb + 1) * 128
                            ],
                            ow_all[:, h, mb * 512 : (mb + 1) * 512],
                            start=(h == 0),
                            stop=(h == n_heads - 1),
                        )
                    nc.scalar.copy(osb[:, mb * 512 : (mb + 1) * 512], ops)
                # one batched row-block store via the gpsimd SWDGE queue
                nc.gpsimd.dma_start(
                    out_d[p_i * 128 : (p_i + 1) * 128, :], osb
                )
            osb_pool.release()
            ow_pool.release()
            att_pool.release()
            mm_ps.release()
            pv_ps.release()
            st_ps.release()
    nc.finalize()
    return nc


def make_core_inputs(x_b, Qw, Qb, Kw, Kb, Vw, Vb, Ow, seq, d_model, n_heads):
    """Host-side prep of one core's input map.

    x_b: [seq, d_model] fp32.  Qw/Kw/Vw: [n_heads, d_model, 128].
    Qb/Kb/Vb: [n_heads, 128].  Ow: [n_heads, 128, d_model].
    """
    nt = d_model // 128
    nhd = n_heads * DH

    # xT as [128(m_in), nt, seq]
    xTr = x_b.T.reshape(nt, 128, seq).transpose(1, 0, 2)
    xt8 = np.ascontiguousarray((xTr * SX).astype(FP8))
    qw = np.ascontiguousarray(
        (Qw * SW).reshape(n_heads, nt, 128, 128).transpose(2, 0, 1, 3).astype(FP8)
    )
    kw = np.ascontiguousarray(
        (Kw * SW).reshape(n_heads, nt, 128, 128).transpose(2, 0, 1, 3).astype(FP8)
    )
    # v weights as [128(m_in), nt, (h d)]
    vw_nat = (
        Vw.transpose(1, 0, 2)
        .reshape(d_model, nhd)
        .reshape(nt, 128, nhd)
        .transpose(1, 0, 2)
    )
    vwk = np.ascontiguousarray(
        np.concatenate([vw_nat, xTr[:, :, 0:128]], axis=2).astype(BF16)
    )
    vw8 = np.ascontiguousarray((vw_nat * SWV).astype(FP8))
    ow = np.ascontiguousarray(Ow.transpose(1, 0, 2).astype(BF16))
    qb = np.ascontiguousarray((Qb * SX * SW).T.astype(np.float32))
    kb = np.ascontiguousarray((Kb * SX * SW).T.astype(np.float32))
    vb = np.ascontiguousarray(
        np.broadcast_to(Vb.reshape(1, nhd), (128, nhd)).astype(BF16)
    )
    r = np.arange(128, dtype=np.int64)[:, None]
    u = np.arange(512, dtype=np.int64)[None, :]
    mask = (r <= u).astype(BF16)
    return {
        "xt8": xt8,
        "vwk": vwk,
        "vw8": vw8,
        "qw": qw,
        "kw": kw,
        "ow": ow,
        "qb": qb,
        "kb": kb,
        "vb": vb,
        "mask": mask,
    }


_NC_CACHE = None


def kernel(**inputs):
    global _NC_CACHE
    from concourse.bass_utils import run_bass_kernel_spmd

    x = np.asarray(inputs["x"], np.float32)
    Q_w = np.asarray(inputs["Q_w"], np.float32)
    Q_b = np.asarray(inputs["Q_b"], np.float32)
    K_w = np.asarray(inputs["K_w"], np.float32)
    K_b = np.asarray(inputs["K_b"], np.float32)
    V_w = np.asarray(inputs["V_w"], np.float32)
    V_b = np.asarray(inputs["V_b"], np.float32)
    O_w = np.asarray(inputs["O_w"], np.float32)
    O_b = np.asarray(inputs["O_b"], np.float32)

    B, seq, d_model = x.shape

    if _NC_CACHE is None:
        _NC_CACHE = build_program(seq=seq, d_model=d_model, n_heads=NH_LOC)
    nc = _NC_CACHE

    in_maps = []
    for c in range(N_CORES):
        b = c // 2
        g = c % 2
        hs = slice(g * NH_LOC, (g + 1) * NH_LOC)
        in_maps.append(
            make_core_inputs(
                x[b], Q_w[hs], Q_b[hs], K_w[hs], K_b[hs], V_w[hs], V_b[hs],
                O_w[hs], seq, d_model, NH_LOC,
            )
        )

    res = run_bass_kernel_spmd(nc, in_maps, core_ids=list(range(N_CORES)))
    out = np.empty((B, seq, d_model), np.float32)
    for b in range(B):
        out[b] = res.results[2 * b]["out"] + res.results[2 * b + 1]["out"] + O_b[None, :]
    return out
